# revision 38
# baseline (speedup 1.0000x reference)
"""Multi-head attention + residual + LayerNorm on 8 Trainium2 NeuronCores.

Sharding: core c in 0..7 handles batch b = c//4 and query-row quarter
r = c%4 (rows 512r..512r+512 of S=2048), with ALL 16 heads.  key/value
are replicated per batch (host-side staging); each core computes the
full-sequence K^T and V projections itself — measured collectives on
this stack cost ~130us per 2MB AllGather, far more than the redundant
PE work, and the local pipeline keeps the PE clock warm.

v5 = 288us (vs 542us fp32r baseline; v2 bf16 408us; v3 fused-exp 347us;
v4 fp8-everywhere 330us — fp8 U matmuls reverted: they starved the PE
into HAM cold-clock, making the exp-bound attention phase slower):
  - host stages x^T (pre-transposed); x and Wq/Wk/Wv in fp8e4, the
    rest bf16: no PE transposes, FWL weight loads, 4x fewer DMA bytes
  - QKV projections run fp8 DoubleRow matmuls (2 adjacent k-chunk
    slices per instruction, 2x rate); attention stays bf16
  - K^T, V_aug, Q^T, ctx all SBUF-resident (no DRAM roundtrips)
  - all PSUM matmul tiles are [128, 1024] 2-bank tiles (tag mm2 ring 2
    + softmax accumulators ut ring 4 = 8 banks): projections pair two
    512-col accumulators per tile and evacuate with ONE wide ACT op;
    attention computes both heads' scores into one tile and exps them
    with ONE 1024-wide ACTIVATE (the ACT 352-cycle/instr overhead was
    pacing the attention phase at 1440ns/chunk vs PE's ~1000ns)
  - softmax denominator broadcast via DRAM-bounce DMA (partition-
    stride-0 read) instead of a PE matmul: normalize is entirely off
    the PE critical path, so pairs pipeline without stalls
  - attention software pipeline crosses pair boundaries (U^T matmuls
    of chunk c issue during chunk c+1, last chunk drains into the next
    pair's first chunk)
  - bulk DMAs ride separate engine queues so the Q-projection feed is
    not queued behind the 4MB K/V loads
  - ~36 warmup matmuls + a dummy exp during the DMA preamble warm the
    PE clock (HAM) and preload the ACT exp table
  - LayerNorm: residual+bo folded on host, normalization via one ACT
    op with per-partition scale/bias; gamma/beta applied only when
    they are non-trivial (checked on host, separate compiled variant)

Accumulations stay fp32 in PSUM; softmax reciprocal / LN stats fp32.
"""

import sys

if "/opt/trn_rl_repo" not in sys.path:
    sys.path.insert(0, "/opt/trn_rl_repo")

import ml_dtypes
import numpy as np

import concourse.bacc as bacc
import concourse.bass as bass
import concourse.mybir as mybir
import concourse.tile as tile
from concourse.bass import ds, ts
from concourse.bass_utils import run_bass_kernel_spmd

FP32R = mybir.dt.float32r
FP32 = mybir.dt.float32
BF16 = mybir.dt.bfloat16
FP8 = mybir.dt.float8e4
AF = mybir.ActivationFunctionType
ALU = mybir.AluOpType
DR = mybir.MatmulPerfMode.DoubleRow
BFNP = ml_dtypes.bfloat16
FP8NP = ml_dtypes.float8_e4m3
# exp(s/8 - EXP_SHIFT) keeps softmax weights inside fp8e4 range (max 240);
# the constant shift cancels exactly in the normalize ratio.
EXP_SHIFT = -3.0

N_CORES = 8
B = 2
S = 2048
D = 1024
H = 16
DK = 64
P = 128

SL = S // 4  # 512 local query rows per core
KC = D // P  # 8 contraction chunks over d_model
SQ = SL // P  # 4 sq subchunks of 128 (per 512-row block)
CH = S // P  # 16 sk chunks
PAIRS = H // 2  # 8 head pairs
NB = 4  # row quarters
EPS = 1e-5

_NC_CACHE = {}


def build_nc(apply_gb: bool):
    nc = bacc.Bacc(num_devices=N_CORES)

    xqt_d = nc.dram_tensor("xqt", [D, SL], FP8, kind="ExternalInput")
    xkt_d = nc.dram_tensor("xkt", [D, S], FP8, kind="ExternalInput")
    xvt_d = nc.dram_tensor("xvt", [D, S], FP8, kind="ExternalInput")
    res_d = nc.dram_tensor("resg", [SL, D], FP32, kind="ExternalInput")
    wq_d = nc.dram_tensor("wq", [D, D], FP8, kind="ExternalInput")
    wk_d = nc.dram_tensor("wk", [D, D], FP8, kind="ExternalInput")
    wv_d = nc.dram_tensor("wv", [D, D], FP8, kind="ExternalInput")
    wo_d = nc.dram_tensor("wo", [D, D], BF16, kind="ExternalInput")
    bq_d = nc.dram_tensor("bq", [D], FP32, kind="ExternalInput")
    bk_d = nc.dram_tensor("bk", [D], FP32, kind="ExternalInput")
    bv_d = nc.dram_tensor("bv", [D], FP32, kind="ExternalInput")
    gam_d = nc.dram_tensor("gam", [D], FP32, kind="ExternalInput")
    bet_d = nc.dram_tensor("bet", [D], FP32, kind="ExternalInput")

    ones_d = nc.dram_tensor("ones", [P, 64], FP32R, kind="ExternalInput")
    y_d = nc.dram_tensor("y", [SL, D], FP32, kind="ExternalOutput")
    # scratch for the softmax-denominator partition broadcast
    zsc_d = nc.dram_tensor("zsc", [PAIRS * 2 * SL], FP32R)

    wq_r = wq_d.rearrange("(c q) m -> q c m", q=P)
    wk_r = wk_d.rearrange("(c q) m -> q c m", q=P)
    wv_r = wv_d.rearrange("(c q) m -> q c m", q=P)
    wo_r = wo_d.rearrange("(c q) m -> q c m", q=P)
    xqt_r = xqt_d.rearrange("(c q) s -> q c s", q=P)
    xkt_r = xkt_d.rearrange("(c q) s -> q c s", q=P)
    xvt_r = xvt_d.rearrange("(c q) s -> q c s", q=P)

    with tile.TileContext(nc) as tc:
        with (
            tc.tile_pool(name="consts", bufs=1) as consts,
            tc.tile_pool(name="big", bufs=1) as big,
            tc.tile_pool(name="xvp", bufs=3) as xvp,
            tc.tile_pool(name="wcol", bufs=3) as wcol,
            tc.tile_pool(name="wres", bufs=1) as wres,
            tc.tile_pool(name="etp", bufs=3) as etp,
            tc.tile_pool(name="normp", bufs=2) as normp,
            tc.tile_pool(name="outp", bufs=2) as outp,
            tc.tile_pool(name="small", bufs=2) as small,
            tc.tile_pool(name="ps", bufs=1, space="PSUM") as ps,
        ):
            # ---- constants + early DMAs for the Q projection ----
            bq_sb = consts.tile([P, KC], FP32)
            nc.sync.dma_start(bq_sb[:], bq_d.rearrange("(m q) -> q m", q=P))
            bk_sb = consts.tile([P, KC], FP32)
            nc.sync.dma_start(bk_sb[:], bk_d.rearrange("(m q) -> q m", q=P))
            xqT = big.tile([P, KC, SL], FP8, tag="xqT")
            nc.sync.dma_start(xqT[:, 0:4, :], xqt_r[:, 0:4, :])
            nc.gpsimd.dma_start(xqT[:, 4:8, :], xqt_r[:, 4:8, :])
            # bulk loads split across queues so they stream in parallel;
            # nothing else may sit in front of these on gpsimd — the
            # broadcast-bias loads go after the K projection instead
            xkT = big.tile([P, KC, S], FP8, tag="xkT")
            nc.scalar.dma_start(xkT[:, 0:4, :], xkt_r[:, 0:4, :])
            nc.gpsimd.dma_start(xkT[:, 4:8, :], xkt_r[:, 4:8, :])
            wv_sb = wres.tile([P, KC, D], FP8, tag="wres8")
            nc.scalar.dma_start(wv_sb[:], wv_r[:])

            def bcast_load(src, tag, dt):
                t = consts.tile([P, D], dt, tag=tag)
                ap = bass.AP(tensor=src, offset=0, ap=[[0, P], [1, D]])
                nc.gpsimd.dma_start(out=t[:], in_=ap)
                return t
            eps_t = consts.tile([P, 1], FP32)
            nc.vector.memset(eps_t[:], EPS)
            on64 = consts.tile([P, 64], FP32R)
            nc.sync.dma_start(on64[:], ones_d[:])

            # ---- PE clock warmup + ACT exp table preload (runs during
            # the input DMA preamble; results are never read) ----
            warm = consts.tile([P, P], BF16)
            nc.vector.memset(warm[:], 0.001)
            wx1 = consts.tile([P, 1], FP32)
            nc.vector.memset(wx1[:], 0.0)
            wxo = consts.tile([P, 1], BF16)
            nc.scalar.activation(out=wxo[:], in_=wx1[:], func=AF.Exp, scale=0.125)
            for _ in range(52):
                pw = ps.tile([P, 2, SL], FP32, tag="mm2", bufs=2)
                nc.tensor.matmul(pw[:, 0, 0:P], warm[:], warm[:], start=True, stop=True)

            # ---- Q^T projection (own rows): qt[q, m, s] = Q^T ----
            qt = big.tile([P, KC, SL], BF16, tag="qt")
            for m in range(KC):
                wq_t = wcol.tile([P, KC, P], FP8, tag="wcol")
                nc.sync.dma_start(wq_t[:], wq_r[:, :, ts(m, P)])
                pq = ps.tile([P, 2, SL], FP32, tag="mm2", bufs=2)
                for k in range(0, KC, 2):
                    nc.tensor.matmul(
                        pq[:, 0, :],
                        wq_t[:, k : k + 2, :],
                        xqT[:, k : k + 2, :],
                        start=(k == 0),
                        stop=(k == KC - 2),
                        perf_mode=DR,
                    )
                nc.scalar.activation(
                    out=qt[:, m, :], in_=pq[:, 0, :], func=AF.Identity,
                    bias=bq_sb[:, m : m + 1],
                )

            # ---- K^T projection (full sequence, SBUF-resident) ----
            ktf = big.tile([P, KC, S], BF16, tag="ktf")
            for m in range(KC):
                wk_t = wcol.tile([P, KC, P], FP8, tag="wcol")
                nc.sync.dma_start(wk_t[:], wk_r[:, :, ts(m, P)])
                for g in range(2):
                    pk = ps.tile([P, 2, SL], FP32, tag="mm2", bufs=2)
                    for k in range(0, KC, 2):
                        for q2 in range(2):
                            nc.tensor.matmul(
                                pk[:, q2, :],
                                wk_t[:, k : k + 2, :],
                                xkT[:, k : k + 2, ds((g * 2 + q2) * SL, SL)],
                                start=(k == 0),
                                stop=(k == KC - 2),
                                perf_mode=DR,
                            )
                    # alternate evacuation between ACT and DVE so neither
                    # engine paces the fp8 matmul stream
                    if g == 0:
                        nc.scalar.activation(
                            out=ktf[:, m, ds(g * 1024, 1024)],
                            in_=pk[:].rearrange("q a s -> q (a s)"),
                            func=AF.Identity,
                            bias=bk_sb[:, m : m + 1],
                        )
                    else:
                        nc.vector.tensor_scalar_add(
                            ktf[:, m, ds(g * 1024, 1024)],
                            pk[:].rearrange("q a s -> q (a s)"),
                            bk_sb[:, m : m + 1],
                        )

            # broadcast-bias loads ride gpsimd after the critical preamble
            bv_b = bcast_load(bv_d, "bv_b", FP32)
            if apply_gb:
                gam_b = bcast_load(gam_d, "gam_b", FP32)
                bet_b = bcast_load(bet_d, "bet_b", FP32)

            # ---- V projection (full sequence, pair-augmented, SBUF) ----
            # vf[q, sc, pair, 130] = [V_even 64 | 1 | V_odd 64 | 1] bf16
            vf = big.tile([P, CH, PAIRS, 130], BF16, tag="vf")
            nc.vector.memset(vf[:, :, :, 64:65], 1.0)
            nc.vector.memset(vf[:, :, :, 129:130], 1.0)
            for sc in range(CH):
                xv_t = xvp.tile([P, KC, P], FP8, tag="xv")
                nc.sync.dma_start(xv_t[:], xvt_r[:, :, ts(sc, P)])
                pv = ps.tile([P, 2, SL], FP32, tag="mm2", bufs=2)
                for k in range(0, KC, 2):
                    for half in range(2):
                        nc.tensor.matmul(
                            pv[:, half, :],
                            xv_t[:, k : k + 2, :],
                            wv_sb[:, k : k + 2, ds(half * 512, 512)],
                            start=(k == 0),
                            stop=(k == KC - 2),
                            perf_mode=DR,
                        )
                for half in range(2):
                    vdst = vf[:, sc, ds(half * 4, 4), :].rearrange(
                        "q pl (j e) -> q pl j e", e=65
                    )
                    nc.vector.tensor_tensor(
                        vdst[:, :, :, 0:64],
                        pv[:, half, :].rearrange("q (pl j e) -> q pl j e", pl=4, j=2),
                        bv_b[:, ds(half * 512, 512)].rearrange(
                            "q (pl j e) -> q pl j e", pl=4, j=2
                        ),
                        ALU.add,
                    )

            # wo and the residual rows load during attention on idle queues
            wo_sb = wres.tile([P, KC, D], BF16, tag="wres")
            nc.scalar.dma_start(wo_sb[:], wo_r[:])
            res_ts = []
            for i in range(SQ):
                res_t = outp.tile([P, D], FP32, tag="res", bufs=4, name=f"res_{i}")
                nc.gpsimd.dma_start(res_t[:], res_d[ts(i, P), :])
                res_ts.append(res_t)

            # ---- attention ----
            ctx = big.tile([P, PAIRS, SL], BF16, tag="ctx")

            def emit_normalize(p_, uA, uB, fast=False):
                # rows 0..63 of ut / row 64 -> ctx[:, p_, :].  The
                # denominator reciprocal is broadcast to 64 partitions by
                # a DRAM bounce (stride-0 partition read) — no PE, no PSUM.
                # The last pair uses a PE broadcast instead: the bounce
                # latency (~12us) would gate the output projection.
                for j, ut in enumerate((uA, uB)):
                    # broadcast the RAW denominators to 64 partitions first,
                    # then reciprocal across 64 DVE lanes (a single-lane
                    # [1,512] reciprocal costs 3.3us; 64-lane costs ~0.25us)
                    zrow = normp.tile([P, SL], FP32R, tag="rec")
                    nc.vector.tensor_copy(zrow[64:65, :], ut[64:65, :])
                    zbc = normp.tile([P, SL], FP32R, tag="zbc")
                    if fast:
                        bcp = ps.tile([P, 2, SL], FP32, tag="mm2", bufs=2)
                        nc.tensor.matmul(
                            bcp[0:64, 0, :],
                            on64[64:65, :],
                            zrow[64:65, :],
                            start=True,
                            stop=True,
                        )
                        nc.vector.tensor_copy(zbc[0:64, :], bcp[0:64, 0, :])
                    else:
                        off = (p_ * 2 + j) * SL
                        nc.sync.dma_start(
                            bass.AP(tensor=zsc_d, offset=off, ap=[[0, 1], [1, SL]]),
                            zrow[64:65, :],
                        )
                        nc.sync.dma_start(
                            zbc[0:64, :],
                            bass.AP(tensor=zsc_d, offset=off, ap=[[0, 64], [1, SL]]),
                        )
                    bcs = normp.tile([P, SL], FP32R, tag="bcs")
                    with nc.allow_low_precision(
                        reason="float32r is bit-identical to float32 in SBUF"
                    ):
                        nc.vector.reciprocal(out=bcs[0:64, :], in_=zbc[0:64, :])
                    if j == 0:
                        nc.vector.tensor_tensor(
                            ctx[0:64, p_, :], ut[0:64, :], bcs[0:64, :], ALU.mult
                        )
                    else:
                        ctmp = normp.tile([P, SL], BF16, tag="ctmp")
                        nc.vector.tensor_tensor(
                            ctmp[0:64, :], ut[0:64, :], bcs[0:64, :], ALU.mult
                        )
                        # partition shift 0-63 -> 64-127 via SBUF-SBUF DMA
                        nc.gpsimd.dma_start(ctx[64:128, p_, :], ctmp[0:64, :])

            pend = None
            norm_pend = None
            for p in range(PAIRS):
                utA = ps.tile([P, SL], FP32, tag="ut", bufs=4)
                utB = ps.tile([P, SL], FP32, tag="ut", bufs=4)
                for c in range(CH):
                    st2 = ps.tile([P, 2, SL], FP32, tag="mm2", bufs=2)
                    for j in range(2):
                        nc.tensor.matmul(
                            st2[:, j, :],
                            ktf[ds(j * 64, 64), p, ts(c, P)],
                            qt[ds(j * 64, 64), p, :],
                            start=True,
                            stop=True,
                        )
                    et2 = etp.tile([P, 2, SL], BF16, tag="et")
                    nc.scalar.activation(
                        out=et2[:], in_=st2[:], func=AF.Exp, scale=0.125
                    )
                    if pend is not None:
                        pp_, pc, pets, puA, puB = pend
                        for j, ut in enumerate((puA, puB)):
                            nc.tensor.matmul(
                                ut[:65, :],
                                vf[:, pc, pp_, ds(j * 65, 65)],
                                pets[:, j, :],
                                start=(pc == 0),
                                stop=(pc == CH - 1),
                            )
                    if c == 2 and norm_pend is not None:
                        emit_normalize(*norm_pend)
                        norm_pend = None
                    pend = (p, c, et2, utA, utB)
                norm_pend = (p, utA, utB)
            pp_, pc, pets, puA, puB = pend
            for j, ut in enumerate((puA, puB)):
                nc.tensor.matmul(
                    ut[:65, :],
                    vf[:, pc, pp_, ds(j * 65, 65)],
                    pets[:, j, :],
                    start=False,
                    stop=True,
                )
            emit_normalize(*norm_pend, fast=True)

            # ---- output projection + residual(+bo) + LayerNorm ----
            for i in range(SQ):
                res_t = res_ts[i]
                po = ps.tile([P, 2, SL], FP32, tag="mm2", bufs=2)
                for n in range(2):
                    for pp2 in range(PAIRS):
                        nc.tensor.matmul(
                            po[:, n, :],
                            ctx[:, pp2, ts(i, P)],
                            wo_sb[:, pp2, ds(n * 512, 512)],
                            start=(pp2 == 0),
                            stop=(pp2 == PAIRS - 1),
                        )
                orow = outp.tile([P, D], FP32, tag="orow")
                nc.vector.tensor_tensor(
                    orow[:], po[:].rearrange("q a s -> q (a s)"), res_t[:], ALU.add
                )
                stats = small.tile([P, 2, 6], FP32, tag="stats")
                nc.vector.bn_stats(stats[:, 0, :], orow[:, 0:512])
                nc.vector.bn_stats(stats[:, 1, :], orow[:, 512:1024])
                mv = small.tile([P, 2], FP32, tag="mv")
                nc.vector.bn_aggr(mv[:], stats[:])
                std = small.tile([P, 1], FP32, tag="std")
                nc.scalar.activation(
                    out=std[:], in_=mv[:, 1:2], func=AF.Sqrt, bias=eps_t[:], scale=1.0
                )
                rstd = small.tile([P, 1], FP32, tag="rstd")
                nc.vector.reciprocal(out=rstd[:], in_=std[:])
                nmr = small.tile([P, 1], FP32, tag="nmr")
                nc.vector.tensor_scalar(
                    out=nmr[:], in0=mv[:, 0:1], scalar1=rstd[:], scalar2=-1.0,
                    op0=ALU.mult, op1=ALU.mult,
                )
                yt = outp.tile([P, D], FP32, tag="yt")
                nc.scalar.activation(
                    out=yt[:], in_=orow[:], func=AF.Identity,
                    bias=nmr[:], scale=rstd[:],
                )
                if apply_gb:
                    nc.vector.tensor_tensor(yt[:], yt[:], gam_b[:], ALU.mult)
                    nc.vector.tensor_tensor(yt[:], yt[:], bet_b[:], ALU.add)
                # split the store across three queues to shrink the drain
                nc.sync.dma_start(y_d[ts(i, P), 0:384], yt[:, 0:384])
                nc.scalar.dma_start(y_d[ts(i, P), 384:704], yt[:, 384:704])
                nc.gpsimd.dma_start(y_d[ts(i, P), 704:1024], yt[:, 704:1024])

    nc.compile()
    return nc


def get_nc(apply_gb: bool):
    key = ("nc", apply_gb)
    if key not in _NC_CACHE:
        _NC_CACHE[key] = build_nc(apply_gb)
    return _NC_CACHE[key]


def kernel(
    query,
    key,
    value,
    Wq,
    bq,
    Wk,
    bk,
    Wv,
    bv,
    Wo,
    bo,
    ln_gamma,
    ln_beta,
    _trace=False,
    _trace_cores=None,
):
    query = np.ascontiguousarray(np.asarray(query, dtype=np.float32))
    key = np.ascontiguousarray(np.asarray(key, dtype=np.float32))
    value = np.ascontiguousarray(np.asarray(value, dtype=np.float32))
    bo_f = np.asarray(bo, np.float32)
    gam_f = np.ascontiguousarray(np.asarray(ln_gamma, np.float32))
    bet_f = np.ascontiguousarray(np.asarray(ln_beta, np.float32))
    apply_gb = not (
        np.all(gam_f == np.float32(1.0)) and np.all(bet_f == np.float32(0.0))
    )
    shared = {
        "wq": np.ascontiguousarray(np.asarray(Wq, np.float32).astype(FP8NP)),
        "wk": np.ascontiguousarray(np.asarray(Wk, np.float32).astype(FP8NP)),
        "wv": np.ascontiguousarray(np.asarray(Wv, np.float32).astype(FP8NP)),
        "wo": np.ascontiguousarray(np.asarray(Wo, np.float32).astype(BFNP)),
        "bq": np.ascontiguousarray(np.asarray(bq, np.float32)),
        "bk": np.ascontiguousarray(np.asarray(bk, np.float32)),
        "bv": np.ascontiguousarray(np.asarray(bv, np.float32)),
        "gam": gam_f,
        "bet": bet_f,
        "ones": np.ones((P, 64), dtype=np.float32),
    }
    kT = [np.ascontiguousarray(key[b].T.astype(FP8NP)) for b in range(B)]
    vT = [np.ascontiguousarray(value[b].T.astype(FP8NP)) for b in range(B)]
    in_maps = []
    for c in range(N_CORES):
        b, r = divmod(c, NB)
        rows = slice(r * SL, (r + 1) * SL)
        xq_rows = query[b, rows, :]
        m = dict(shared)
        m["xqt"] = np.ascontiguousarray(xq_rows.T.astype(FP8NP))
        m["xkt"] = kT[b]
        m["xvt"] = vT[b]
        m["resg"] = np.ascontiguousarray(xq_rows + bo_f[None, :])
        in_maps.append(m)

    nc = get_nc(apply_gb)
    res = run_bass_kernel_spmd(
        nc,
        in_maps,
        list(range(N_CORES)),
        trace=_trace,
        trace_cores=_trace_cores,
    )
    out = np.empty((B, S, D), dtype=np.float32)
    for c in range(N_CORES):
        b, r = divmod(c, NB)
        out[b, r * SL : (r + 1) * SL, :] = res.results[c]["y"]
    if _trace:
        return out, res
    return out


# revision 43
# speedup vs baseline: 1.0305x; 1.0305x over previous
"""Multi-head attention + residual + LayerNorm on 8 Trainium2 NeuronCores.

Sharding: core c in 0..7 handles batch b = c//4 and query-row quarter
r = c%4 (rows 512r..512r+512 of S=2048), with ALL 16 heads.  key/value
are replicated per batch (host-side staging); each core computes the
full-sequence K^T and V projections itself — measured collectives on
this stack cost ~130us per 2MB AllGather, far more than the redundant
PE work, and the local pipeline keeps the PE clock warm.

v5 = 288us (vs 542us fp32r baseline; v2 bf16 408us; v3 fused-exp 347us;
v4 fp8-everywhere 330us — fp8 U matmuls reverted: they starved the PE
into HAM cold-clock, making the exp-bound attention phase slower):
  - host stages x^T (pre-transposed); x and Wq/Wk/Wv in fp8e4, the
    rest bf16: no PE transposes, FWL weight loads, 4x fewer DMA bytes
  - QKV projections run fp8 DoubleRow matmuls (2 adjacent k-chunk
    slices per instruction, 2x rate); attention stays bf16
  - K^T, V_aug, Q^T, ctx all SBUF-resident (no DRAM roundtrips)
  - all PSUM matmul tiles are [128, 1024] 2-bank tiles (tag mm2 ring 2
    + softmax accumulators ut ring 4 = 8 banks): projections pair two
    512-col accumulators per tile and evacuate with ONE wide ACT op;
    attention computes both heads' scores into one tile and exps them
    with ONE 1024-wide ACTIVATE (the ACT 352-cycle/instr overhead was
    pacing the attention phase at 1440ns/chunk vs PE's ~1000ns)
  - softmax denominator broadcast via DRAM-bounce DMA (partition-
    stride-0 read) instead of a PE matmul: normalize is entirely off
    the PE critical path, so pairs pipeline without stalls
  - attention software pipeline crosses pair boundaries (U^T matmuls
    of chunk c issue during chunk c+1, last chunk drains into the next
    pair's first chunk)
  - bulk DMAs ride separate engine queues so the Q-projection feed is
    not queued behind the 4MB K/V loads
  - ~36 warmup matmuls + a dummy exp during the DMA preamble warm the
    PE clock (HAM) and preload the ACT exp table
  - LayerNorm: residual+bo folded on host, normalization via one ACT
    op with per-partition scale/bias; gamma/beta applied only when
    they are non-trivial (checked on host, separate compiled variant)

Accumulations stay fp32 in PSUM; softmax reciprocal / LN stats fp32.
"""

import sys

if "/opt/trn_rl_repo" not in sys.path:
    sys.path.insert(0, "/opt/trn_rl_repo")

import ml_dtypes
import numpy as np

import concourse.bacc as bacc
import concourse.bass as bass
import concourse.mybir as mybir
import concourse.tile as tile
from concourse.bass import ds, ts
from concourse.bass_utils import run_bass_kernel_spmd

FP32R = mybir.dt.float32r
FP32 = mybir.dt.float32
BF16 = mybir.dt.bfloat16
FP8 = mybir.dt.float8e4
AF = mybir.ActivationFunctionType
ALU = mybir.AluOpType
DR = mybir.MatmulPerfMode.DoubleRow
BFNP = ml_dtypes.bfloat16
FP8NP = ml_dtypes.float8_e4m3
# exp(s/8 - EXP_SHIFT) keeps softmax weights inside fp8e4 range (max 240);
# the constant shift cancels exactly in the normalize ratio.
EXP_SHIFT = -3.0

N_CORES = 8
B = 2
S = 2048
D = 1024
H = 16
DK = 64
P = 128

SL = S // 4  # 512 local query rows per core
KC = D // P  # 8 contraction chunks over d_model
SQ = SL // P  # 4 sq subchunks of 128 (per 512-row block)
CH = S // P  # 16 sk chunks
PAIRS = H // 2  # 8 head pairs
NB = 4  # row quarters
EPS = 1e-5

_NC_CACHE = {}


def build_nc(apply_gb: bool):
    nc = bacc.Bacc(num_devices=N_CORES)

    xqt_d = nc.dram_tensor("xqt", [D, SL], FP8, kind="ExternalInput")
    xkt_d = nc.dram_tensor("xkt", [D, S], FP8, kind="ExternalInput")
    xvt_d = nc.dram_tensor("xvt", [D, S], FP8, kind="ExternalInput")
    res_d = nc.dram_tensor("resg", [SL, D], FP32, kind="ExternalInput")
    wq_d = nc.dram_tensor("wq", [D, D], FP8, kind="ExternalInput")
    wk_d = nc.dram_tensor("wk", [D, D], FP8, kind="ExternalInput")
    wv_d = nc.dram_tensor("wv", [D, D], FP8, kind="ExternalInput")
    wo_d = nc.dram_tensor("wo", [D, D], BF16, kind="ExternalInput")
    bq_d = nc.dram_tensor("bq", [D], FP32, kind="ExternalInput")
    bk_d = nc.dram_tensor("bk", [D], FP32, kind="ExternalInput")
    bv_d = nc.dram_tensor("bv", [D], FP32, kind="ExternalInput")
    gam_d = nc.dram_tensor("gam", [D], FP32, kind="ExternalInput")
    bet_d = nc.dram_tensor("bet", [D], FP32, kind="ExternalInput")

    ones_d = nc.dram_tensor("ones", [P, 64], FP32R, kind="ExternalInput")
    y_d = nc.dram_tensor("y", [SL, D], FP32, kind="ExternalOutput")
    # scratch for the softmax-denominator partition broadcast
    zsc_d = nc.dram_tensor("zsc", [PAIRS * 2 * SL], FP32R)

    wq_r = wq_d.rearrange("(c q) m -> q c m", q=P)
    wk_r = wk_d.rearrange("(c q) m -> q c m", q=P)
    wv_r = wv_d.rearrange("(c q) m -> q c m", q=P)
    wo_r = wo_d.rearrange("(c q) m -> q c m", q=P)
    xqt_r = xqt_d.rearrange("(c q) s -> q c s", q=P)
    xkt_r = xkt_d.rearrange("(c q) s -> q c s", q=P)
    xvt_r = xvt_d.rearrange("(c q) s -> q c s", q=P)

    with tile.TileContext(nc) as tc:
        with (
            tc.tile_pool(name="consts", bufs=1) as consts,
            tc.tile_pool(name="big", bufs=1) as big,
            tc.tile_pool(name="xvp", bufs=3) as xvp,
            tc.tile_pool(name="wcol", bufs=3) as wcol,
            tc.tile_pool(name="wres", bufs=1) as wres,
            tc.tile_pool(name="etp", bufs=3) as etp,
            tc.tile_pool(name="normp", bufs=2) as normp,
            tc.tile_pool(name="outp", bufs=2) as outp,
            tc.tile_pool(name="small", bufs=2) as small,
            tc.tile_pool(name="ps", bufs=1, space="PSUM") as ps,
        ):
            # ---- constants + early DMAs for the Q projection ----
            bq_sb = consts.tile([P, KC], FP32)
            nc.sync.dma_start(bq_sb[:], bq_d.rearrange("(m q) -> q m", q=P))
            bk_sb = consts.tile([P, KC], FP32)
            nc.sync.dma_start(bk_sb[:], bk_d.rearrange("(m q) -> q m", q=P))
            xqT = big.tile([P, KC, SL], FP8, tag="xqT")
            nc.sync.dma_start(xqT[:, 0:4, :], xqt_r[:, 0:4, :])
            nc.gpsimd.dma_start(xqT[:, 4:8, :], xqt_r[:, 4:8, :])
            # bulk loads split across queues so they stream in parallel;
            # nothing else may sit in front of these on gpsimd — the
            # broadcast-bias loads go after the K projection instead
            xkT = big.tile([P, KC, S], FP8, tag="xkT")
            nc.scalar.dma_start(xkT[:, 0:4, :], xkt_r[:, 0:4, :])
            nc.gpsimd.dma_start(xkT[:, 4:8, :], xkt_r[:, 4:8, :])
            wv_sb = wres.tile([P, KC, D], FP8, tag="wres8")
            nc.scalar.dma_start(wv_sb[:], wv_r[:])

            def bcast_load(src, tag, dt):
                t = consts.tile([P, D], dt, tag=tag)
                ap = bass.AP(tensor=src, offset=0, ap=[[0, P], [1, D]])
                nc.gpsimd.dma_start(out=t[:], in_=ap)
                return t
            eps_t = consts.tile([P, 1], FP32)
            nc.vector.memset(eps_t[:], EPS)

            # ---- PE clock warmup + ACT exp table preload (runs during
            # the input DMA preamble; results are never read) ----
            warm = consts.tile([P, P], BF16)
            nc.vector.memset(warm[:], 0.001)
            wx1 = consts.tile([P, 1], FP32)
            nc.vector.memset(wx1[:], 0.0)
            wxo = consts.tile([P, 1], BF16)
            nc.scalar.activation(out=wxo[:], in_=wx1[:], func=AF.Exp, scale=0.125)
            for _ in range(52):
                pw = ps.tile([P, 2, SL], FP32, tag="mm2", bufs=2)
                nc.tensor.matmul(pw[:, 0, 0:P], warm[:], warm[:], start=True, stop=True)

            # ---- Q^T projection (own rows): qt[q, m, s] = Q^T ----
            qt = big.tile([P, KC, SL], BF16, tag="qt")
            for m in range(KC):
                wq_t = wcol.tile([P, KC, P], FP8, tag="wcol")
                nc.sync.dma_start(wq_t[:], wq_r[:, :, ts(m, P)])
                pq = ps.tile([P, 2, SL], FP32, tag="mm2", bufs=2)
                for k in range(0, KC, 2):
                    nc.tensor.matmul(
                        pq[:, 0, :],
                        wq_t[:, k : k + 2, :],
                        xqT[:, k : k + 2, :],
                        start=(k == 0),
                        stop=(k == KC - 2),
                        perf_mode=DR,
                    )
                nc.scalar.activation(
                    out=qt[:, m, :], in_=pq[:, 0, :], func=AF.Identity,
                    bias=bq_sb[:, m : m + 1],
                )

            # ---- K^T projection (full sequence, SBUF-resident) ----
            ktf = big.tile([P, KC, S], BF16, tag="ktf")
            for m in range(KC):
                wk_t = wcol.tile([P, KC, P], FP8, tag="wcol")
                nc.sync.dma_start(wk_t[:], wk_r[:, :, ts(m, P)])
                for g in range(2):
                    pk = ps.tile([P, 2, SL], FP32, tag="mm2", bufs=2)
                    for k in range(0, KC, 2):
                        for q2 in range(2):
                            nc.tensor.matmul(
                                pk[:, q2, :],
                                wk_t[:, k : k + 2, :],
                                xkT[:, k : k + 2, ds((g * 2 + q2) * SL, SL)],
                                start=(k == 0),
                                stop=(k == KC - 2),
                                perf_mode=DR,
                            )
                    # alternate evacuation between ACT and DVE so neither
                    # engine paces the fp8 matmul stream
                    if g == 0:
                        nc.scalar.activation(
                            out=ktf[:, m, ds(g * 1024, 1024)],
                            in_=pk[:].rearrange("q a s -> q (a s)"),
                            func=AF.Identity,
                            bias=bk_sb[:, m : m + 1],
                        )
                    else:
                        nc.vector.tensor_scalar_add(
                            ktf[:, m, ds(g * 1024, 1024)],
                            pk[:].rearrange("q a s -> q (a s)"),
                            bk_sb[:, m : m + 1],
                        )

            # broadcast-bias loads ride gpsimd after the critical preamble;
            # ones (needed only for the tail normalize) loads late too
            on64 = consts.tile([P, 64], FP32R)
            nc.sync.dma_start(on64[:], ones_d[:])
            bv_b = bcast_load(bv_d, "bv_b", FP32)
            if apply_gb:
                gam_b = bcast_load(gam_d, "gam_b", FP32)
                bet_b = bcast_load(bet_d, "bet_b", FP32)

            # ---- V projection (full sequence, pair-augmented, SBUF) ----
            # vf[q, sc, pair, 130] = [V_even 64 | 1 | V_odd 64 | 1] bf16
            vf = big.tile([P, CH, PAIRS, 130], BF16, tag="vf")
            nc.vector.memset(vf[:, :, :, 64:65], 1.0)
            nc.vector.memset(vf[:, :, :, 129:130], 1.0)
            for sc in range(CH):
                xv_t = xvp.tile([P, KC, P], FP8, tag="xv")
                nc.sync.dma_start(xv_t[:], xvt_r[:, :, ts(sc, P)])
                pv = ps.tile([P, 2, SL], FP32, tag="mm2", bufs=2)
                for k in range(0, KC, 2):
                    for half in range(2):
                        nc.tensor.matmul(
                            pv[:, half, :],
                            xv_t[:, k : k + 2, :],
                            wv_sb[:, k : k + 2, ds(half * 512, 512)],
                            start=(k == 0),
                            stop=(k == KC - 2),
                            perf_mode=DR,
                        )
                for half in range(2):
                    vdst = vf[:, sc, ds(half * 4, 4), :].rearrange(
                        "q pl (j e) -> q pl j e", e=65
                    )
                    nc.vector.tensor_tensor(
                        vdst[:, :, :, 0:64],
                        pv[:, half, :].rearrange("q (pl j e) -> q pl j e", pl=4, j=2),
                        bv_b[:, ds(half * 512, 512)].rearrange(
                            "q (pl j e) -> q pl j e", pl=4, j=2
                        ),
                        ALU.add,
                    )

            # wo and the residual rows load during attention on idle queues
            wo_sb = wres.tile([P, KC, D], BF16, tag="wres")
            nc.scalar.dma_start(wo_sb[:], wo_r[:])
            res_ts = []
            for i in range(SQ):
                res_t = outp.tile([P, D], FP32, tag="res", bufs=4, name=f"res_{i}")
                nc.gpsimd.dma_start(res_t[:], res_d[ts(i, P), :])
                res_ts.append(res_t)

            # ---- attention ----
            ctx = big.tile([P, PAIRS, SL], BF16, tag="ctx")

            def emit_normalize(p_, uA, uB, fast=False):
                # rows 0..63 of ut / row 64 -> ctx[:, p_, :].  The
                # denominator reciprocal is broadcast to 64 partitions by
                # a DRAM bounce (stride-0 partition read) — no PE, no PSUM.
                # The last pair uses a PE broadcast instead: the bounce
                # latency (~12us) would gate the output projection.
                for j, ut in enumerate((uA, uB)):
                    bcs = normp.tile([P, SL], FP32R, tag="bcs")
                    if fast:
                        # exposed tail path: broadcast RAW denominators via
                        # PE, then reciprocal across 64 DVE lanes — a
                        # single-lane [1,512] reciprocal costs 3.3us
                        zrow = normp.tile([P, SL], FP32R, tag="rec")
                        nc.vector.tensor_copy(zrow[64:65, :], ut[64:65, :])
                        bcp = ps.tile([P, 2, SL], FP32, tag="mm2", bufs=2)
                        nc.tensor.matmul(
                            bcp[0:64, 0, :],
                            on64[64:65, :],
                            zrow[64:65, :],
                            start=True,
                            stop=True,
                        )
                        with nc.allow_low_precision(
                            reason="float32r is bit-identical to float32 in SBUF"
                        ):
                            nc.vector.reciprocal(
                                out=bcs[0:64, :], in_=bcp[0:64, 0, :]
                            )
                    else:
                        rec = normp.tile([P, SL], FP32R, tag="rec")
                        with nc.allow_low_precision(
                            reason="float32r is bit-identical to float32 in SBUF"
                        ):
                            nc.vector.reciprocal(out=rec[64:65, :], in_=ut[64:65, :])
                        off = (p_ * 2 + j) * SL
                        nc.sync.dma_start(
                            bass.AP(tensor=zsc_d, offset=off, ap=[[0, 1], [1, SL]]),
                            rec[64:65, :],
                        )
                        nc.sync.dma_start(
                            bcs[0:64, :],
                            bass.AP(tensor=zsc_d, offset=off, ap=[[0, 64], [1, SL]]),
                        )
                    if j == 0:
                        nc.vector.tensor_tensor(
                            ctx[0:64, p_, :], ut[0:64, :], bcs[0:64, :], ALU.mult
                        )
                    else:
                        ctmp = normp.tile([P, SL], BF16, tag="ctmp")
                        nc.vector.tensor_tensor(
                            ctmp[0:64, :], ut[0:64, :], bcs[0:64, :], ALU.mult
                        )
                        # partition shift 0-63 -> 64-127 via SBUF-SBUF DMA
                        nc.gpsimd.dma_start(ctx[64:128, p_, :], ctmp[0:64, :])

            pend = None
            norm_pend = None
            for p in range(PAIRS):
                utA = ps.tile([P, SL], FP32, tag="ut", bufs=4)
                utB = ps.tile([P, SL], FP32, tag="ut", bufs=4)
                for c in range(CH):
                    st2 = ps.tile([P, 2, SL], FP32, tag="mm2", bufs=2)
                    for j in range(2):
                        nc.tensor.matmul(
                            st2[:, j, :],
                            ktf[ds(j * 64, 64), p, ts(c, P)],
                            qt[ds(j * 64, 64), p, :],
                            start=True,
                            stop=True,
                        )
                    et2 = etp.tile([P, 2, SL], BF16, tag="et")
                    nc.scalar.activation(
                        out=et2[:], in_=st2[:], func=AF.Exp, scale=0.125
                    )
                    if pend is not None:
                        pp_, pc, pets, puA, puB = pend
                        for j, ut in enumerate((puA, puB)):
                            nc.tensor.matmul(
                                ut[:65, :],
                                vf[:, pc, pp_, ds(j * 65, 65)],
                                pets[:, j, :],
                                start=(pc == 0),
                                stop=(pc == CH - 1),
                            )
                    if c == 2 and norm_pend is not None:
                        emit_normalize(*norm_pend)
                        norm_pend = None
                    pend = (p, c, et2, utA, utB)
                norm_pend = (p, utA, utB)
            pp_, pc, pets, puA, puB = pend
            for j, ut in enumerate((puA, puB)):
                nc.tensor.matmul(
                    ut[:65, :],
                    vf[:, pc, pp_, ds(j * 65, 65)],
                    pets[:, j, :],
                    start=False,
                    stop=True,
                )
            emit_normalize(*norm_pend, fast=True)

            # ---- output projection + residual(+bo) + LayerNorm ----
            for i in range(SQ):
                res_t = res_ts[i]
                po = ps.tile([P, 2, SL], FP32, tag="mm2", bufs=2)
                for n in range(2):
                    for pp2 in range(PAIRS):
                        nc.tensor.matmul(
                            po[:, n, :],
                            ctx[:, pp2, ts(i, P)],
                            wo_sb[:, pp2, ds(n * 512, 512)],
                            start=(pp2 == 0),
                            stop=(pp2 == PAIRS - 1),
                        )
                orow = outp.tile([P, D], FP32, tag="orow")
                nc.vector.tensor_tensor(
                    orow[:], po[:].rearrange("q a s -> q (a s)"), res_t[:], ALU.add
                )
                stats = small.tile([P, 2, 6], FP32, tag="stats")
                nc.vector.bn_stats(stats[:, 0, :], orow[:, 0:512])
                nc.vector.bn_stats(stats[:, 1, :], orow[:, 512:1024])
                mv = small.tile([P, 2], FP32, tag="mv")
                nc.vector.bn_aggr(mv[:], stats[:])
                std = small.tile([P, 1], FP32, tag="std")
                nc.scalar.activation(
                    out=std[:], in_=mv[:, 1:2], func=AF.Sqrt, bias=eps_t[:], scale=1.0
                )
                rstd = small.tile([P, 1], FP32, tag="rstd")
                nc.vector.reciprocal(out=rstd[:], in_=std[:])
                nmr = small.tile([P, 1], FP32, tag="nmr")
                nc.vector.tensor_scalar(
                    out=nmr[:], in0=mv[:, 0:1], scalar1=rstd[:], scalar2=-1.0,
                    op0=ALU.mult, op1=ALU.mult,
                )
                yt = outp.tile([P, D], FP32, tag="yt")
                nc.scalar.activation(
                    out=yt[:], in_=orow[:], func=AF.Identity,
                    bias=nmr[:], scale=rstd[:],
                )
                if apply_gb:
                    nc.vector.tensor_tensor(yt[:], yt[:], gam_b[:], ALU.mult)
                    nc.vector.tensor_tensor(yt[:], yt[:], bet_b[:], ALU.add)
                # split the store across three queues to shrink the drain
                nc.sync.dma_start(y_d[ts(i, P), 0:384], yt[:, 0:384])
                nc.scalar.dma_start(y_d[ts(i, P), 384:704], yt[:, 384:704])
                nc.gpsimd.dma_start(y_d[ts(i, P), 704:1024], yt[:, 704:1024])

    nc.compile()
    return nc


def get_nc(apply_gb: bool):
    key = ("nc", apply_gb)
    if key not in _NC_CACHE:
        _NC_CACHE[key] = build_nc(apply_gb)
    return _NC_CACHE[key]


def kernel(
    query,
    key,
    value,
    Wq,
    bq,
    Wk,
    bk,
    Wv,
    bv,
    Wo,
    bo,
    ln_gamma,
    ln_beta,
    _trace=False,
    _trace_cores=None,
):
    query = np.ascontiguousarray(np.asarray(query, dtype=np.float32))
    key = np.ascontiguousarray(np.asarray(key, dtype=np.float32))
    value = np.ascontiguousarray(np.asarray(value, dtype=np.float32))
    bo_f = np.asarray(bo, np.float32)
    gam_f = np.ascontiguousarray(np.asarray(ln_gamma, np.float32))
    bet_f = np.ascontiguousarray(np.asarray(ln_beta, np.float32))
    apply_gb = not (
        np.all(gam_f == np.float32(1.0)) and np.all(bet_f == np.float32(0.0))
    )
    shared = {
        "wq": np.ascontiguousarray(np.asarray(Wq, np.float32).astype(FP8NP)),
        "wk": np.ascontiguousarray(np.asarray(Wk, np.float32).astype(FP8NP)),
        "wv": np.ascontiguousarray(np.asarray(Wv, np.float32).astype(FP8NP)),
        "wo": np.ascontiguousarray(np.asarray(Wo, np.float32).astype(BFNP)),
        "bq": np.ascontiguousarray(np.asarray(bq, np.float32)),
        "bk": np.ascontiguousarray(np.asarray(bk, np.float32)),
        "bv": np.ascontiguousarray(np.asarray(bv, np.float32)),
        "gam": gam_f,
        "bet": bet_f,
        "ones": np.ones((P, 64), dtype=np.float32),
    }
    kT = [np.ascontiguousarray(key[b].T.astype(FP8NP)) for b in range(B)]
    vT = [np.ascontiguousarray(value[b].T.astype(FP8NP)) for b in range(B)]
    in_maps = []
    for c in range(N_CORES):
        b, r = divmod(c, NB)
        rows = slice(r * SL, (r + 1) * SL)
        xq_rows = query[b, rows, :]
        m = dict(shared)
        m["xqt"] = np.ascontiguousarray(xq_rows.T.astype(FP8NP))
        m["xkt"] = kT[b]
        m["xvt"] = vT[b]
        m["resg"] = np.ascontiguousarray(xq_rows + bo_f[None, :])
        in_maps.append(m)

    nc = get_nc(apply_gb)
    res = run_bass_kernel_spmd(
        nc,
        in_maps,
        list(range(N_CORES)),
        trace=_trace,
        trace_cores=_trace_cores,
    )
    out = np.empty((B, S, D), dtype=np.float32)
    for c in range(N_CORES):
        b, r = divmod(c, NB)
        out[b, r * SL : (r + 1) * SL, :] = res.results[c]["y"]
    if _trace:
        return out, res
    return out
